# revision 5
# baseline (speedup 1.0000x reference)
"""Trainium2 Bass kernel for a chain of 2 invertible-ResNet blocks
(dense MLP 2->256, 4x 256->256, 256->2, ELU, residual) over 1M points.

Strategy: pure data parallel over 8 NeuronCores; points transposed to
[2, N] on host so activations live as [256, FD] tiles (features on
partitions, points on the free dim).  Matmuls run in float32r (full PE
rate).  ELU is computed in 2 instructions per tile:
    ACT:  e  = Exp(y + b_eff)          (PSUM -> SBUF, bias fused)
    DVE:  h'' = max(y, -b_eff) + min(e, 1)   (one custom fused op)
with the resulting constant shift (b_eff - 1) folded into the next
layer's effective bias (precomputed on host).  The residual stream is
accumulated entirely in PSUM:  out = I*x0 + w_out0^T h''_L0 +
w_out1^T h''_L1 + bias, using W01 = w_out0 @ w_in1 to absorb block0's
residual into block1's input projection.
"""

import numpy as np

import concourse.bass as bass
import concourse.tile as tile
from concourse import bacc, mybir
from concourse.bass_utils import run_bass_kernel_spmd
from concourse.dve_spec import Spec, Src0, Src1, C0, C1, maxx, minn
import concourse.dve_ops as dve_ops
from concourse.dve_ops import DveOp

F32 = mybir.dt.float32
F32R = mybir.dt.float32r

NUM_NODES = 2
H = 256
L = 4
D = 2
N_CORES = 8

FD = 1024          # points per chunk (free dim)
HALF = 512         # matmul free-dim (one PSUM bank)


def _register_elu_tail():
    name = "ELU_TAIL_ANT"
    for op in dve_ops.OPS:
        if op.name == name:
            return op
    op = DveOp(
        name,
        Spec(
            body=maxx(Src0, C0) + minn(Src1, C1),
            reference=lambda in0, in1, s0, s1, imm2: (
                np.maximum(in0.astype(np.float32), s0)
                + np.minimum(in1.astype(np.float32), s1)
            ),
        ),
        subdim=False,
        uops_sha={"v3": "b9e41bc1a54edf6f", "v4": "2155f01abd9df135"},
    )
    dve_ops.OPS.append(op)
    dve_ops._SUB_OPCODE_FOR_NAME[name] = (
        dve_ops._CUSTOM_DVE_ROW_BASE + len(dve_ops.OPS) - 1
    )
    dve_ops.CUSTOM_DVE_SPECS[name] = op.spec
    return op


def _effective_params(w_in, b_in, w_hid, b_hid, w_out, b_out):
    """Fold the ELU-tail constant shifts into effective biases (float64)."""
    w_in = w_in.astype(np.float64)
    b_in = b_in.astype(np.float64)
    w_hid = w_hid.astype(np.float64)
    b_hid = b_hid.astype(np.float64)
    w_out = w_out.astype(np.float64)
    b_out = b_out.astype(np.float64)

    b_eff = np.zeros((2 * (1 + L), H))          # per ELU layer
    # block 0
    b_eff[0] = b_in[0]
    c = b_eff[0] - 1.0
    for l in range(L):
        b_eff[1 + l] = b_hid[0, l] + c @ w_hid[0, l]
        c = b_eff[1 + l] - 1.0
    bo0 = b_out[0] + c @ w_out[0]               # [2]
    # block 1 (input = true x1, but x1 is never materialized; its bias
    # contribution rides through bo0)
    b_eff[5] = b_in[1] + bo0 @ w_in[1]
    c = b_eff[5] - 1.0
    for l in range(L):
        b_eff[6 + l] = b_hid[1, l] + c @ w_hid[1, l]
        c = b_eff[6 + l] - 1.0
    bo1 = b_out[1] + c @ w_out[1]               # [2]
    w01 = w_out[0] @ w_in[1]                    # [H, H]
    bo_total = bo0 + bo1                        # [2]

    # pack biases as [128, 20]: column = layer*2 + mtile
    bp = np.zeros((128, 20), np.float32)
    bn = np.zeros((128, 20), np.float32)
    for j in range(10):
        for m in range(2):
            col = b_eff[j, m * 128:(m + 1) * 128]
            bp[:, j * 2 + m] = col.astype(np.float32)
            bn[:, j * 2 + m] = (-col).astype(np.float32)
    return bp, bn, w01.astype(np.float32), bo_total.astype(np.float32)


def _build_program(nsh, unroll, n_iters, repeat=1):
    """Build the SPMD Bass program for one core processing `nsh` points.

    nsh = n_iters * unroll * FD.  When n_iters > 1 a hardware For_i loop
    runs the body (unroll chunks) n_iters times.  `repeat` re-runs the
    whole pass (benchmarking aid; output identical).
    """
    ELU_TAIL = _register_elu_tail()
    nc = bacc.Bacc("TRN2", target_bir_lowering=False, debug=False,
                   num_devices=N_CORES)

    uvT = nc.declare_dram_parameter("uvT", [D, nsh], F32, isOutput=False).ap()
    WIN = nc.declare_dram_parameter("WIN", [2, D, H], F32, isOutput=False).ap()
    W01 = nc.declare_dram_parameter("W01", [H, H], F32, isOutput=False).ap()
    WH = nc.declare_dram_parameter("WH", [8, H, H], F32, isOutput=False).ap()
    WO = nc.declare_dram_parameter("WO", [2, H, D], F32, isOutput=False).ap()
    IDE = nc.declare_dram_parameter("IDE", [D, D], F32, isOutput=False).ap()
    BP = nc.declare_dram_parameter("BP", [128, 20], F32, isOutput=False).ap()
    BN = nc.declare_dram_parameter("BN", [128, 20], F32, isOutput=False).ap()
    BOT = nc.declare_dram_parameter("BOT", [D, 1], F32, isOutput=False).ap()
    outT = nc.declare_dram_parameter("outT", [D, nsh], F32, isOutput=True).ap()

    with tile.TileContext(nc) as tc:
        with (
            tc.tile_pool(name="wpool", bufs=1) as wp,
            tc.tile_pool(name="xpool", bufs=4) as xp,
            tc.tile_pool(name="epool", bufs=3) as ep,
            tc.tile_pool(name="hpool", bufs=6) as hp,
            tc.tile_pool(name="opool", bufs=3) as op,
            tc.tile_pool(name="ypool", bufs=2, space="PSUM") as yp,
            tc.tile_pool(name="yopool", bufs=2, space="PSUM") as yop,
        ):
            # ---- persistent weights/biases (loaded once) ----
            win = [wp.tile([D, H], F32R, tag=f"win{i}", name=f"win{i}") for i in range(2)]
            for i in range(2):
                nc.gpsimd.dma_start(out=win[i], in_=WIN[i])
            w01 = [wp.tile([128, H], F32R, tag=f"w01k{k}", name=f"w01k{k}") for k in range(2)]
            for k in range(2):
                nc.gpsimd.dma_start(out=w01[k], in_=W01[k * 128:(k + 1) * 128, :])
            wh = [[wp.tile([128, H], F32R, tag=f"wh{j}k{k}", name=f"wh{j}k{k}") for k in range(2)]
                  for j in range(8)]
            for j in range(8):
                for k in range(2):
                    nc.gpsimd.dma_start(out=wh[j][k],
                                        in_=WH[j, k * 128:(k + 1) * 128, :])
            wo = [[wp.tile([128, D], F32R, tag=f"wo{i}k{k}", name=f"wo{i}k{k}") for k in range(2)]
                  for i in range(2)]
            for i in range(2):
                for k in range(2):
                    nc.gpsimd.dma_start(out=wo[i][k],
                                        in_=WO[i, k * 128:(k + 1) * 128, :])
            ide = wp.tile([D, D], F32R, tag="ide")
            nc.gpsimd.dma_start(out=ide, in_=IDE)
            bp = wp.tile([128, 20], F32, tag="bp")
            nc.gpsimd.dma_start(out=bp, in_=BP)
            bn = wp.tile([128, 20], F32, tag="bn")
            nc.gpsimd.dma_start(out=bn, in_=BN)
            bot = wp.tile([D, 1], F32, tag="bot")
            nc.gpsimd.dma_start(out=bot, in_=BOT)

            def chunk_body(sl):
                """sl: column slice (bass.ds or python slice) of FD points."""
                x0 = xp.tile([D, FD], F32R)
                nc.gpsimd.dma_start(out=x0, in_=uvT[:, sl])

                yo = yop.tile([D, FD], F32)
                for nh in range(FD // HALF):
                    cs = slice(nh * HALF, (nh + 1) * HALF)
                    nc.tensor.matmul(yo[:, cs], ide, x0[:, cs],
                                     start=True, stop=False)

                h = [None, None]
                for j in range(10):                     # ELU layers
                    newh = [None, None]
                    for m in range(2):
                        mcs = slice(m * 128, (m + 1) * 128)
                        y = yp.tile([128, FD], F32)
                        for nh in range(FD // HALF):
                            cs = slice(nh * HALF, (nh + 1) * HALF)
                            if j == 0:
                                nc.tensor.matmul(y[:, cs], win[0][:, mcs],
                                                 x0[:, cs], start=True, stop=True)
                            elif j == 5:
                                nc.tensor.matmul(y[:, cs], win[1][:, mcs],
                                                 x0[:, cs], start=True, stop=False)
                                nc.tensor.matmul(y[:, cs], w01[0][:, mcs],
                                                 h[0][:, cs], start=False, stop=False)
                                nc.tensor.matmul(y[:, cs], w01[1][:, mcs],
                                                 h[1][:, cs], start=False, stop=True)
                            else:
                                jh = j - 1 if j < 5 else j - 2  # 0..3, 4..7
                                nc.tensor.matmul(y[:, cs], wh[jh][0][:, mcs],
                                                 h[0][:, cs], start=True, stop=False)
                                nc.tensor.matmul(y[:, cs], wh[jh][1][:, mcs],
                                                 h[1][:, cs], start=False, stop=True)
                        col = j * 2 + m
                        e = ep.tile([128, FD], F32)
                        nc.scalar.activation(e, y, mybir.ActivationFunctionType.Exp,
                                             bias=bp[:, col:col + 1])
                        hn = hp.tile([128, FD], F32R)
                        nc.vector._custom_dve(ELU_TAIL, out=hn, in0=y, in1=e,
                                              s0=bn[:, col:col + 1], s1=1.0)
                        newh[m] = hn
                    h = newh
                    if j == 4 or j == 9:               # block output proj
                        i = 0 if j == 4 else 1
                        last = (j == 9)
                        for nh in range(FD // HALF):
                            cs = slice(nh * HALF, (nh + 1) * HALF)
                            nc.tensor.matmul(yo[:, cs], wo[i][0], h[0][:, cs],
                                             start=False, stop=False)
                            nc.tensor.matmul(yo[:, cs], wo[i][1], h[1][:, cs],
                                             start=False, stop=last)
                xo = op.tile([D, FD], F32)
                nc.scalar.activation(xo, yo, mybir.ActivationFunctionType.Identity,
                                     bias=bot[:, 0:1])
                nc.sync.dma_start(out=outT[:, sl], in_=xo)

            for _rep in range(repeat):
                if n_iters == 1:
                    for u in range(unroll):
                        chunk_body(slice(u * FD, (u + 1) * FD))
                else:
                    step = unroll * FD
                    with tc.For_i(0, n_iters * step, step,
                                  hint_engines=(mybir.EngineType.PE,)) as it:
                        for u in range(unroll):
                            chunk_body(bass.ds(it + u * FD, FD))

    nc.finalize()
    return nc


_PROGRAM_CACHE = {}


def _get_program(nsh, unroll, n_iters, repeat=1):
    key = (nsh, unroll, n_iters, repeat)
    if key not in _PROGRAM_CACHE:
        _PROGRAM_CACHE[key] = _build_program(nsh, unroll, n_iters, repeat)
    return _PROGRAM_CACHE[key]


def _run(uv, w_in, b_in, w_hid, b_hid, w_out, b_out, unroll, n_iters):
    n = uv.shape[0]
    nsh = n // N_CORES
    assert nsh == n_iters * unroll * FD

    bp, bn, w01, bo_total = _effective_params(w_in, b_in, w_hid, b_hid,
                                              w_out, b_out)
    base = {
        "WIN": np.ascontiguousarray(w_in.astype(np.float32)),
        "W01": w01,
        "WH": np.ascontiguousarray(w_hid.reshape(8, H, H).astype(np.float32)),
        "WO": np.ascontiguousarray(w_out.astype(np.float32)),
        "IDE": np.eye(D, dtype=np.float32),
        "BP": bp,
        "BN": bn,
        "BOT": bo_total.reshape(D, 1).astype(np.float32),
    }
    in_maps = []
    for c in range(N_CORES):
        shard = uv[c * nsh:(c + 1) * nsh]
        m = dict(base)
        m["uvT"] = np.ascontiguousarray(shard.T.astype(np.float32))
        in_maps.append(m)

    nc = _get_program(nsh, unroll, n_iters)
    res = run_bass_kernel_spmd(nc, in_maps, core_ids=list(range(N_CORES)))
    outs = [res.results[c]["outT"].T for c in range(N_CORES)]
    return np.ascontiguousarray(np.concatenate(outs, axis=0)).astype(np.float32)


def kernel(uv, w_in, b_in, w_hid, b_hid, w_out, b_out):
    n = uv.shape[0]
    nsh = n // N_CORES
    # pick loop shape: prefer hardware loop with unrolled body
    n_chunks = nsh // FD
    if n_chunks >= 16 and n_chunks % 8 == 0:
        unroll, n_iters = 8, n_chunks // 8
    else:
        unroll, n_iters = n_chunks, 1
    return _run(uv, w_in, b_in, w_hid, b_hid, w_out, b_out, unroll, n_iters)
